# revision 16
# baseline (speedup 1.0000x reference)
"""MoE grouped-linear (ragged matmul + bias) on 8 TRN2 NeuronCores.

Expert-parallel sharding: core e computes tokens of expert e:
    out_e = X_e[cap, 2048] @ W_e[2048, 8192] + bias
Tokens are pre-sorted by expert (contiguous groups), so the "all-to-all"
is a free host-side slice/concat. No on-device collectives.

Compute runs in fp8e4m3 with MatmulPerfMode.DoubleRow, which on TRN2
measures ~4x the f32r/bf16 matmul rate at this shape (118ns vs 234ns
per MM, and a DoubleRow MM contracts 2 k-tiles). Plain fp8 fails the
2e-2 accuracy gate (~4e-2), so X and W are split hi/lo:

    X*16 = Xhi + Xlo   (fp8 + fp8 residual)
    W*64 = Whi + Wlo
    X@W ~= (Xhi@Whi + Xlo@Whi + Xhi@Wlo) / 1024     [drop Xlo@Wlo]

Three fp8 passes at 4x = ~1.33x over one f32r pass in theory, measured
better because DoubleRow also halves the MM instruction count. Max rel
err ~1.3e-3 (vs 1.6e-4 for f32r baseline, gate is 2e-2).

Per-core kernel: Xhi/Xlo fully resident in SBUF (gpsimd DMA queue),
Whi/Wlo streamed per-ni in host-contiguous 1MB chunks (sync + scalar
queues), 24 DoubleRow MMs per (ni, mi) accumulate into one PSUM bank,
eviction = ACT descale (1/1024) + DVE bias add + DMA out (vector
queue), all overlapped with the PE stream.
"""

import numpy as np

E, IN, OUT = 8, 2048, 8192
P = 128
NTILE = 512
KT = IN // P          # 16 k-tiles
NT = OUT // NTILE     # 16 n-chunks
SX = 16.0
SW = 64.0
DESCALE = 1.0 / (SX * SW)

_cache = {}


def _build(cap, reps=1, mode="full", nb=4):
    import concourse.mybir as mybir
    import concourse.tile as tile
    from concourse import bacc

    from concourse.bass import ds

    f8 = mybir.dt.float8e4
    f32 = mybir.dt.float32
    MT = cap // P
    DR = mybir.MatmulPerfMode.DoubleRow
    NB = nb                # ni per hardware-loop iteration
    NIT = NT // NB         # hardware-loop trip count per rep

    nc = bacc.Bacc(None, target_bir_lowering=False, debug=False)
    with tile.TileContext(nc) as tc:
        with tc.tile_pool(name="dram", bufs=1, space="DRAM") as dram:
            # xhi/xlo[mi, p, k, j] = q8(X[mi*P + j, k*P + p] * 16), per-mi
            # contiguous 256KB slices; whi/wlo[ni, p, k, n] =
            # q8(W[k*P + p, ni*NTILE + n] * 64), per-ni contiguous 1MB
            # chunks, padded with 4 wrapped chunks for the tail prefetch
            xhi_d = dram.tile((MT, P, KT, P), f8, kind="ExternalInput")
            xlo_d = dram.tile((MT, P, KT, P), f8, kind="ExternalInput")
            whi_d = dram.tile((NT + 4, P, KT, NTILE), f8, kind="ExternalInput")
            wlo_d = dram.tile((NT + 4, P, KT, NTILE), f8, kind="ExternalInput")
            bias_d = dram.tile((P, OUT), f32, kind="ExternalInput")
            out_d = dram.tile((P, MT, OUT), f32, kind="ExternalOutput")

            with tc.tile_pool(name="res", bufs=1) as res_pool, \
                 tc.tile_pool(name="wres", bufs=1) as w_pool, \
                 tc.tile_pool(name="ev", bufs=4) as ev_pool, \
                 tc.tile_pool(name="ev2", bufs=4) as ev2_pool, \
                 tc.tile_pool(name="acc", bufs=6, space="PSUM") as ps_pool:
                # NB resident W-slot pairs, software-pipelined: slot j
                # holds chunk NB*itm + j and reloads chunk NB*(itm+1) + j
                # right after its last matmul of the iteration. reps are
                # flattened into ONE hardware loop (itm = it % NIT) so the
                # loop body is the only PE instruction stream — nesting a
                # reps loop around it defeats IRAM residency.
                whi_sb = [w_pool.tile([P, KT, NTILE], f8, tag=f"whi{j}",
                                      name=f"whi_sb{j}")
                          for j in range(NB)]
                wlo_sb = [w_pool.tile([P, KT, NTILE], f8, tag=f"wlo{j}",
                                      name=f"wlo_sb{j}")
                          for j in range(NB)]
                for j in range(NB):
                    nc.sync.dma_start(whi_sb[j][:], whi_d[j])
                    nc.sync.dma_start(wlo_sb[j][:], wlo_d[j])

                xhi_sb = [res_pool.tile([P, KT, P], f8, tag=f"xh{mi}",
                                        name=f"xhi_sb{mi}")
                          for mi in range(MT)]
                xlo_sb = [res_pool.tile([P, KT, P], f8, tag=f"xl{mi}",
                                        name=f"xlo_sb{mi}")
                          for mi in range(MT)]
                bias_sb = res_pool.tile([P, OUT], f32)
                nc.gpsimd.dma_start(xhi_sb[0][:], xhi_d[0])
                nc.gpsimd.dma_start(xlo_sb[0][:], xlo_d[0])
                nc.gpsimd.dma_start(bias_sb[:], bias_d[:])
                for mi in range(1, MT):
                    nc.gpsimd.dma_start(xhi_sb[mi][:], xhi_d[mi])
                    nc.gpsimd.dma_start(xlo_sb[mi][:], xlo_d[mi])

                with tc.For_i(0, NIT * reps, 1) as it:
                    itm = it % NIT if reps > 1 else it
                    for jn in range(NB):
                        whi, wlo = whi_sb[jn], wlo_sb[jn]
                        col = itm * (NB * NTILE) + jn * NTILE
                        for mi in range(MT):
                            ps = ps_pool.tile([P, NTILE], f32, tag="ps",
                                              name=f"ps{jn}_{mi}")
                            for j in range(KT // 2):
                                sl = slice(2 * j, 2 * j + 2)
                                nc.tensor.matmul(
                                    ps[:], lhsT=xhi_sb[mi][:, sl, :],
                                    rhs=whi[:, sl, :], perf_mode=DR,
                                    start=(j == 0), stop=False)
                            for j in range(KT // 2):
                                sl = slice(2 * j, 2 * j + 2)
                                nc.tensor.matmul(
                                    ps[:], lhsT=xhi_sb[mi][:, sl, :],
                                    rhs=wlo[:, sl, :], perf_mode=DR,
                                    start=False, stop=False)
                            for j in range(KT // 2):
                                sl = slice(2 * j, 2 * j + 2)
                                nc.tensor.matmul(
                                    ps[:], lhsT=xlo_sb[mi][:, sl, :],
                                    rhs=whi[:, sl, :], perf_mode=DR,
                                    start=False, stop=(j == KT // 2 - 1))
                            o = ev_pool.tile([P, NTILE], f32, tag="o",
                                             name=f"o{jn}_{mi}")
                            nc.scalar.mul(out=o[:], in_=ps[:], mul=DESCALE)
                            o2 = ev2_pool.tile([P, NTILE], f32, tag="o2",
                                               name=f"o2{jn}_{mi}")
                            nc.vector.tensor_add(
                                out=o2[:], in0=o[:],
                                in1=bias_sb[:, ds(col, NTILE)])
                            nc.scalar.dma_start(
                                out_d[:, mi, ds(col, NTILE)], o2[:])
                        # slot jn done for this iteration; prefetch its
                        # chunk for the next one
                        nc.sync.dma_start(whi[:], whi_d[itm * NB + (NB + jn)])
                        nc.sync.dma_start(wlo[:], wlo_d[itm * NB + (NB + jn)])
    nc.compile()
    names = dict(xhi=xhi_d.name, xlo=xlo_d.name, whi=whi_d.name,
                 wlo=wlo_d.name, bias=bias_d.name, out=out_d.name)
    return nc, names


def _get(cap, reps=1, mode="full"):
    key = (cap, reps, mode)
    if key not in _cache:
        _cache[key] = _build(cap, reps, mode)
    return _cache[key]


def _f8():
    import concourse.mybir as mybir
    return mybir.dt.np(mybir.dt.float8e4)


def _q8(a, f8):
    # TRN fp8e4 tops out at +-240 (not OCP's 448); clip before cast
    return np.clip(a, -240.0, 240.0).astype(f8)


def _split8(a, scale, f8):
    s = (a * np.float32(scale)).astype(np.float32)
    hi = _q8(s, f8)
    lo = _q8(s - hi.astype(np.float32), f8)
    return hi, lo


def kernel(inputs, weight, group_sizes, bias):
    from concourse.bass_utils import run_bass_kernel_spmd

    f8 = _f8()
    M = inputs.shape[0]
    gs = np.asarray(group_sizes, dtype=np.int64)
    # per-token expert id exactly as the reference's jnp.repeat(...,
    # total_repeat_length=M): truncate or pad with the last expert id
    ids = np.repeat(np.arange(E), gs)
    ids = ids[:M] if len(ids) >= M else np.concatenate(
        [ids, np.full(M - len(ids), E - 1)])
    counts = np.bincount(ids, minlength=E)
    starts = np.concatenate([[0], np.cumsum(counts)])[:E]

    cap = max(P, int(-(-counts.max() // P) * P))
    MT = cap // P
    nc, names = _get(cap)

    x = np.ascontiguousarray(inputs, dtype=np.float32)
    w = np.asarray(weight, dtype=np.float32)
    bias_rep = np.ascontiguousarray(
        np.broadcast_to(np.asarray(bias, np.float32), (P, OUT)))

    in_maps = []
    for e in range(E):
        xe = x[starts[e]:starts[e] + counts[e]]
        if xe.shape[0] < cap:
            xe = np.concatenate(
                [xe, np.zeros((cap - xe.shape[0], IN), np.float32)])
        xhi, xlo = _split8(xe, SX, f8)
        # [cap, IN] -> (MT, P, KT, P): xt[mi, p, k, j] = X[mi*P+j, k*P+p]
        def xt(a):
            return np.ascontiguousarray(
                a.reshape(MT, P, KT, P).transpose(0, 3, 2, 1))
        whi, wlo = _split8(w[e], SW, f8)
        # [IN, OUT] -> (NT+4, P, KT, NTILE): wt[ni, p, k, n] =
        # W[k*P+p, ni*NTILE+n]; 4 wrapped chunks pad the tail prefetch
        def wt(a):
            t = a.reshape(KT, P, NT, NTILE).transpose(2, 1, 0, 3)
            return np.ascontiguousarray(np.concatenate([t, t[:4]], axis=0))
        in_maps.append({names["xhi"]: xt(xhi), names["xlo"]: xt(xlo),
                        names["whi"]: wt(whi), names["wlo"]: wt(wlo),
                        names["bias"]: bias_rep})

    res = run_bass_kernel_spmd(nc, in_maps, core_ids=list(range(E)))
    out = np.empty((M, OUT), dtype=np.float32)
    for e in range(E):
        oe = res.results[e][names["out"]]          # (P, MT, OUT)
        oe = oe.transpose(1, 0, 2).reshape(cap, OUT)
        out[starts[e]:starts[e] + counts[e]] = oe[:counts[e]]
    return out
